# revision 10
# baseline (speedup 1.0000x reference)
"""CBConv2d (change-based conv) Trainium2 kernel, 8-core SPMD.

Reference semantics (B=1, C=64, H=W=512, 3x3 SAME conv):
  changed = any_c(|inp - prev_input| > 0.1)            # [H, W]
  dilated = maxpool3x3(changed)                        # [H, W]
  out     = dilated ? (conv2d(inp, w) + bias) : prev_output
Sharding: H split across 8 cores (64 rows each), halos materialized on host.

Per-core device pipeline (4 tiles of 16 output rows):
  - all HBM I/O in bf16 (inp/prev_input/prev_output in, out back; host
    up-casts to fp32). Conv runs on TensorE in bf16 with fp32 PSUM, rows
    paired (r, r+8) across partition halves so every epilogue op runs on
    128 partitions.
  - change mask: DVE subtract + square, Pool (GPSIMD) fused
    (d^2 - thr^2) -> max(,0) into an fp8 indicator; per-pixel change count
    AND the H-dilation come from 5 fp8 DoubleRow matmuls (256-deep
    contraction) with 3-wide banded 1/64 weights; W-dilation is 2 small DVE
    adds; PE ones-matmuls broadcast the dilated count across partitions into
    PSUM; one copy_predicated per row-pair merges conv over prev_output.

Mask exactness notes: inputs are bf16-rounded and the indicator is
fp8-rounded (values below ~2^-9 flush to 0), so pixels whose |diff| sits
within a fraction of a percent of the threshold can flip vs the fp32
reference. A flipped pixel only affects the output if its entire 3x3
neighborhood has no other changed pixel; with this data distribution
(~95% changed) the expected number of affected output pixels is ~1e-7.
bf16 output rounding adds ~0.4% rel err, well under the 2e-2 gate.
"""
import numpy as np
import ml_dtypes

import concourse.bass as bass
import concourse.mybir as mybir
import concourse.tile as tile
from concourse import bacc
from concourse.bass_utils import run_bass_kernel_spmd

F32 = mybir.dt.float32
BF16 = mybir.dt.bfloat16
F8 = mybir.dt.float8e4
BF = ml_dtypes.bfloat16
F8NP = mybir.dt.np(mybir.dt.float8e4)

C = 64          # channels
H = W = 512     # spatial
NCORES = 8
RPC = H // NCORES          # rows per core (64)
R = 16                     # output rows per tile
NT = RPC // R              # tiles per core (4)
NPAD = R + 2               # padded rows per tile (18)
G = 10                     # rows per partition-group (overlapping: lower=0..9, upper=8..17)
WP = W + 2                 # padded width (514)
THR2 = float(np.float32(0.1) * np.float32(0.1))

_cached = {}


def build_nc(loop_iters: int = 0, variant: str = "full"):
    """Build the per-core Bass program. loop_iters>0 wraps the whole pipeline
    in a For_i loop that re-executes it (for slope-based timing).

    variant tokens (comma-joined) progressively strip stages for debugging:
      nosel   - plain copy instead of copy_predicated
      nomb    - also skip mask-broadcast matmuls
      nodil   - also skip W-dilation + dil1 DMA
      nocnt   - also skip count matmuls
      noind   - also skip indicator ops (pure conv kernel)
      noconv  - skip conv matmuls + evac (mask pipeline only; copy prev->out)
      actrelu - indicator relu on ACT (Square+Relu) instead of Pool
      nodr    - bf16 cnt via 10 plain matmuls instead of fp8 DoubleRow
      nopoolhint - drop Pool from For_i hint_engines
    """
    has_ind = "noind" not in variant
    has_cnt = has_ind and "nocnt" not in variant
    has_dil = has_cnt and "nodil" not in variant
    has_mb = has_dil and "nomb" not in variant
    has_sel = has_mb and "nosel" not in variant
    has_conv = "noconv" not in variant
    use_pool = "actrelu" not in variant
    use_dr = "nodr" not in variant
    ind_dt = F8 if use_dr else BF16

    nc = bacc.Bacc("TRN2", target_bir_lowering=False, debug=False,
                   enable_asserts=True, num_devices=NCORES)

    xin = nc.dram_tensor("xin", [NT, 128, G, WP], BF16, kind="ExternalInput")
    pin = nc.dram_tensor("pin", [NT, 128, G, WP], BF16, kind="ExternalInput")
    pout = nc.dram_tensor("pout", [NT, 128, 8 * W], BF16, kind="ExternalInput")
    seldr = nc.dram_tensor("seldr", [128, 5, 2, R], F8, kind="ExternalInput")
    sel10 = nc.dram_tensor("sel10", [128, G * R], BF16, kind="ExternalInput")
    sel2x = nc.dram_tensor("sel2x", [2, 128], BF16, kind="ExternalInput")
    biasv = nc.dram_tensor("biasv", [128, 1], F32, kind="ExternalInput")
    wtbd = nc.dram_tensor("wtbd", [128, 9 * 128], BF16, kind="ExternalInput")
    outd = nc.dram_tensor("out", [NT, 128, 8 * W], BF16, kind="ExternalOutput")

    with tile.TileContext(nc) as tc:
        with tc.tile_pool(name="consts", bufs=1) as cpool, \
             tc.tile_pool(name="io", bufs=3) as iopool, \
             tc.tile_pool(name="mask", bufs=2) as mpool, \
             tc.tile_pool(name="cnt", bufs=2, space="PSUM") as cntpool, \
             tc.tile_pool(name="conv", bufs=4, space="PSUM") as convpool, \
             tc.tile_pool(name="mb", bufs=2, space="PSUM") as mbpool:

            seldrt = cpool.tile([128, 5, 2, R], F8)
            sel10t = cpool.tile([128, G * R], BF16)
            sel2xt = cpool.tile([2, 128], BF16)
            biast = cpool.tile([128, 1], F32)
            wtbdt = cpool.tile([128, 9 * 128], BF16)
            negthr = cpool.tile([128, 1], F32)
            nc.sync.dma_start(out=seldrt[:], in_=seldr[:])
            nc.sync.dma_start(out=sel10t[:], in_=sel10[:])
            nc.sync.dma_start(out=sel2xt[:], in_=sel2x[:])
            nc.sync.dma_start(out=biast[:], in_=biasv[:])
            nc.sync.dma_start(out=wtbdt[:], in_=wtbd[:])
            nc.vector.memset(negthr[:], -THR2)

            def emit_tile(t):
                xt = iopool.tile([128, G, WP], BF16, tag="xt")
                pt = iopool.tile([128, G, WP], BF16, tag="pt")
                pvt = iopool.tile([128, 8 * W], BF16, tag="pvt")
                nc.sync.dma_start(out=xt[:], in_=xin[t])
                nc.sync.dma_start(out=pt[:], in_=pin[t])
                nc.sync.dma_start(out=pvt[:], in_=pout[t])

                dil1 = None
                if has_ind:
                    # --- change indicator: max((x-p)^2 - thr^2, 0) as fp8 ---
                    d = mpool.tile([128, G, WP], BF16, tag="d")
                    nc.vector.tensor_tensor(out=d[:], in0=xt[:], in1=pt[:],
                                            op=mybir.AluOpType.subtract)
                    ind = mpool.tile([128, G, WP], ind_dt, tag="ind")
                    if use_pool:
                        nc.vector.tensor_tensor(out=d[:], in0=d[:], in1=d[:],
                                                op=mybir.AluOpType.mult)
                        nc.gpsimd.tensor_scalar(out=ind[:], in0=d[:],
                                                scalar1=THR2, scalar2=0.0,
                                                op0=mybir.AluOpType.subtract,
                                                op1=mybir.AluOpType.max)
                    else:
                        nc.scalar.activation(d[:], d[:],
                                             mybir.ActivationFunctionType.Square)
                        nc.scalar.activation(ind[:], d[:],
                                             mybir.ActivationFunctionType.Relu,
                                             bias=negthr[:])

                if has_cnt:
                    # --- change count + H-dilation via banded matmuls ---
                    # fp8 DoubleRow: each MM contracts 2 row-slots x 128
                    # partitions (rows 8,9 counted twice -- harmless, only
                    # nonzero-ness is used). Band weights are 1/64 so dil
                    # values stay well inside fp8 range.
                    cnt = cntpool.tile([R, W], F32, tag="cnt")
                    if use_dr:
                        for k in range(5):
                            nc.tensor.matmul(
                                cnt[:], seldrt[:, k],
                                ind[:, 2 * k:2 * k + 2, 1:1 + W],
                                start=(k == 0), stop=(k == 4),
                                perf_mode=mybir.MatmulPerfMode.DoubleRow)
                    else:
                        for k in range(G):
                            nc.tensor.matmul(
                                cnt[:], sel10t[:, k * R:(k + 1) * R],
                                ind[:, k, 1:1 + W],
                                start=(k == 0), stop=(k == G - 1))

                if has_dil:
                    # --- W-dilation on [R, W+2] ---
                    hs = mpool.tile([R, WP], F32, tag="hs")
                    nc.vector.memset(hs[:], 0.0)
                    nc.vector.tensor_copy(out=hs[:, 1:W + 1], in_=cnt[:])
                    t1 = mpool.tile([R, W + 1], F32, tag="t1")
                    nc.vector.tensor_tensor(out=t1[:], in0=hs[:, 0:W + 1],
                                            in1=hs[:, 1:WP],
                                            op=mybir.AluOpType.add)
                    dil = mpool.tile([R, W], BF16, tag="dil")
                    nc.vector.tensor_tensor(out=dil[:], in0=t1[:, 0:W],
                                            in1=hs[:, 2:WP],
                                            op=mybir.AluOpType.add)
                    dil1 = mpool.tile([2, 8 * W], BF16, tag="dil1")
                    nc.scalar.dma_start(out=dil1[:], in_=dil[:])

                conv_sb = iopool.tile([128, 8 * W], BF16, tag="conv_sb")
                for j in range(8):
                    sl = slice(j * W, (j + 1) * W)
                    if has_conv:
                        # --- conv pair: rows (j, j+8) -> one PSUM bank ---
                        cb = convpool.tile([128, W], F32, tag="cb")
                        # block-diag lhsT [[W,0],[0,W]] computes BOTH halves
                        # of the pair in one 128-contraction MM: partitions
                        # 0:64 = group0 row j+dh -> out 0:64; partitions
                        # 64:128 = group1 row j+dh (= row 8+j+dh) -> 64:128.
                        taps = [(dh, dw) for dh in range(3) for dw in range(3)]
                        for i, (dh, dw) in enumerate(taps):
                            ti = dh * 3 + dw
                            nc.tensor.matmul(
                                cb[:],
                                wtbdt[:, ti * 128:(ti + 1) * 128],
                                xt[:, j + dh, dw:dw + W],
                                start=(i == 0), stop=(i == len(taps) - 1))
                        # --- evacuate conv + bias (fp32 psum -> bf16) ---
                        nc.scalar.activation(
                            conv_sb[:, sl], cb[:],
                            mybir.ActivationFunctionType.Identity,
                            bias=biast[:])

                for j in range(8):
                    sl = slice(j * W, (j + 1) * W)
                    if has_mb:
                        # --- broadcast dilated counts for rows (j, j+8) ---
                        mb = mbpool.tile([128, W], F32, tag="mb")
                        nc.tensor.matmul(mb[:], sel2xt[:],
                                         dil1[:, j * W:(j + 1) * W],
                                         start=True, stop=True)

                    # --- merge conv over prev_output ---
                    if has_sel and has_conv:
                        nc.vector.copy_predicated(
                            pvt[:, sl], mb[:].bitcast(mybir.dt.int32),
                            conv_sb[:, sl])
                    elif has_conv:
                        nc.vector.tensor_copy(out=pvt[:, sl],
                                              in_=conv_sb[:, sl])

                nc.scalar.dma_start(out=outd[t], in_=pvt[:])

            hints = [mybir.EngineType.PE, mybir.EngineType.DVE,
                     mybir.EngineType.Activation, mybir.EngineType.SP]
            if use_pool and has_ind and "nopoolhint" not in variant:
                hints.insert(3, mybir.EngineType.Pool)
            unroll = 1
            for tok in variant.split(","):
                if tok.startswith("u") and tok[1:].isdigit():
                    unroll = int(tok[1:])
            if loop_iters > 0:
                def body(iv):
                    for t in range(NT):
                        emit_tile(t)
                if unroll > 1:
                    tc.For_i_unrolled(0, loop_iters, 1, body, unroll)
                else:
                    with tc.For_i(0, loop_iters, 1, hint_engines=tuple(hints),
                                  staggered_reset="sr" in variant.split(",")):
                        body(0)
            else:
                for t in range(NT):
                    emit_tile(t)

    nc.compile()
    return nc


def host_prep(inp, prev_input, prev_output, weight, bias):
    """Build per-core in_maps."""
    inp = np.asarray(inp)
    prev_input = np.asarray(prev_input)
    prev_output = np.asarray(prev_output)
    weight = np.asarray(weight)
    bias = np.asarray(bias)

    xpad = np.zeros((C, H + 2, WP), dtype=BF)
    ppad = np.zeros((C, H + 2, WP), dtype=BF)
    xpad[:, 1:H + 1, 1:W + 1] = inp[0].astype(BF)
    ppad[:, 1:H + 1, 1:W + 1] = prev_input[0].astype(BF)

    # sel bands: group0 handles padded rows p=0..9 (slot k=p), group1
    # p=10..17 (slot k=p-8). band(k)[u] = 1/64 if k-2 <= u <= k, 0 <= u < R.
    selA = np.zeros((G, R), dtype=np.float32)
    selB = np.zeros((G, R), dtype=np.float32)
    for p in range(0, 10):
        for u in range(max(0, p - 2), min(R, p + 1)):
            selA[p, u] = 1.0 / 64
    for p in range(10, NPAD):
        for u in range(max(0, p - 2), min(R, p + 1)):
            selB[p - 8, u] = 1.0 / 64
    seldr = np.empty((128, 5, 2, R), dtype=F8NP)
    for k in range(G):
        seldr[:64, k // 2, k % 2] = selA[k].astype(F8NP)
        seldr[64:, k // 2, k % 2] = selB[k].astype(F8NP)
    sel10 = np.empty((128, G * R), dtype=BF)
    sel10[:64] = selA.astype(BF).reshape(1, G * R)
    sel10[64:] = selB.astype(BF).reshape(1, G * R)

    wtbd = np.zeros((128, 9 * 128), dtype=BF)
    for dh in range(3):
        for dw in range(3):
            ti = dh * 3 + dw
            wtap_ = weight[:, :, dh, dw].T.astype(BF)   # [ci, co]
            wtbd[0:64, ti * 128:ti * 128 + 64] = wtap_
            wtbd[64:128, ti * 128 + 64:(ti + 1) * 128] = wtap_

    sel2x = np.zeros((2, 128), dtype=BF)
    sel2x[0, :64] = 1
    sel2x[1, 64:] = 1
    biasv = np.tile(bias.astype(np.float32).reshape(-1, 1), (2, 1))  # [128,1]

    in_maps = []
    for c in range(NCORES):
        r0 = c * RPC

        def slab(pad):
            s = np.empty((NT, 128, G, WP), dtype=BF)
            for t in range(NT):
                rows = pad[:, r0 + 16 * t: r0 + 16 * t + NPAD, :]  # [C,18,WP]
                s[t, :64] = rows[:, 0:10]
                s[t, 64:] = rows[:, 8:18]
            return s

        po = prev_output[0][:, r0:r0 + RPC, :].reshape(C, NT, 2, 8, W)
        po = po.transpose(1, 2, 0, 3, 4).reshape(NT, 128, 8 * W)
        po = np.ascontiguousarray(po).astype(BF)

        in_maps.append({
            "xin": slab(xpad), "pin": slab(ppad), "pout": po,
            "seldr": seldr, "sel10": sel10, "sel2x": sel2x,
            "biasv": biasv, "wtbd": wtbd,
        })
    return in_maps


def host_post(results):
    """Reassemble [NCORES] x [NT, 128, 8*W] bf16 -> [1, C, H, W] fp32."""
    out = np.empty((1, C, H, W), dtype=np.float32)
    for c, res in enumerate(results):
        o = res["out"].astype(np.float32)
        o = o.reshape(NT, 2, C, 8, W).transpose(2, 0, 1, 3, 4)
        out[0, :, c * RPC:(c + 1) * RPC, :] = o.reshape(C, RPC, W)
    return out


def kernel(inp, prev_input, prev_output, weight, bias):
    if "nc" not in _cached:
        _cached["nc"] = build_nc(0)
    nc = _cached["nc"]
    in_maps = host_prep(inp, prev_input, prev_output, weight, bias)
    res = run_bass_kernel_spmd(nc, in_maps, core_ids=list(range(NCORES)))
    return host_post(res.results)


if __name__ == "__main__":
    rng = np.random.default_rng(0)
    inp = rng.standard_normal((1, C, H, W), dtype=np.float32)
    prev_input = inp + 0.05 * rng.standard_normal((1, C, H, W), dtype=np.float32)
    prev_output = rng.standard_normal((1, C, H, W), dtype=np.float32)
    weight = (0.05 * rng.standard_normal((C, C, 3, 3))).astype(np.float32)
    bias = rng.standard_normal(C).astype(np.float32)
    out = kernel(inp=inp, prev_input=prev_input, prev_output=prev_output,
                 weight=weight, bias=bias)
    print("out", out.shape, out.dtype, float(np.abs(out).mean()))


# revision 13
# speedup vs baseline: 1.3311x; 1.3311x over previous
"""CBConv2d (change-based conv) Trainium2 kernel, 8-core SPMD.

Reference semantics (B=1, C=64, H=W=512, 3x3 SAME conv):
  changed = any_c(|inp - prev_input| > 0.1)            # [H, W]
  dilated = maxpool3x3(changed)                        # [H, W]
  out     = dilated ? (conv2d(inp, w) + bias) : prev_output
Sharding: H split across 8 cores (64 rows each), halos materialized on host.

Per-core device pipeline (4 tiles of 16 output rows):
  - all HBM I/O in bf16 (inp/prev_input/prev_output in, out back; host
    up-casts to fp32). Conv runs on TensorE in bf16 with fp32 PSUM, rows
    paired (r, r+8) across partition halves so every epilogue op runs on
    128 partitions.
  - change mask: DVE subtract + square, Pool (GPSIMD) fused
    (d^2 - thr^2) -> max(,0) into an fp8 indicator; per-pixel change count
    AND the H-dilation come from 5 fp8 DoubleRow matmuls (256-deep
    contraction) with 3-wide banded 1/64 weights; W-dilation is 2 small DVE
    adds; PE ones-matmuls broadcast the dilated count across partitions into
    PSUM; one copy_predicated per row-pair merges conv over prev_output.

Mask exactness notes: inputs are bf16-rounded and the indicator is
fp8-rounded (values below ~2^-9 flush to 0), so pixels whose |diff| sits
within a fraction of a percent of the threshold can flip vs the fp32
reference. A flipped pixel only affects the output if its entire 3x3
neighborhood has no other changed pixel; with this data distribution
(~95% changed) the expected number of affected output pixels is ~1e-7.
bf16 output rounding adds ~0.4% rel err, well under the 2e-2 gate.
"""
import numpy as np
import ml_dtypes

import concourse.bass as bass
import concourse.mybir as mybir
import concourse.tile as tile
from concourse import bacc
from concourse.bass_utils import run_bass_kernel_spmd

F32 = mybir.dt.float32
BF16 = mybir.dt.bfloat16
F8 = mybir.dt.float8e4
BF = ml_dtypes.bfloat16
F8NP = mybir.dt.np(mybir.dt.float8e4)

C = 64          # channels
H = W = 512     # spatial
NCORES = 8
RPC = H // NCORES          # rows per core (64)
R = 16                     # output rows per tile
NT = RPC // R              # tiles per core (4)
NPAD = R + 2               # padded rows per tile (18)
G = 10                     # rows per partition-group (overlapping: lower=0..9, upper=8..17)
WP = W + 2                 # padded width (514)
THR2 = float(np.float32(0.1) * np.float32(0.1))

_cached = {}


def build_nc(loop_iters: int = 0, variant: str = "full"):
    """Build the per-core Bass program. loop_iters>0 wraps the whole pipeline
    in a For_i loop that re-executes it (for slope-based timing).

    variant tokens (comma-joined) progressively strip stages for debugging:
      nosel   - plain copy instead of copy_predicated
      nomb    - also skip mask-broadcast matmuls
      nodil   - also skip W-dilation + dil1 DMA
      nocnt   - also skip count matmuls
      noind   - also skip indicator ops (pure conv kernel)
      noconv  - skip conv matmuls + evac (mask pipeline only; copy prev->out)
      actrelu - indicator relu on ACT (Square+Relu) instead of Pool
      nodr    - bf16 cnt via 10 plain matmuls instead of fp8 DoubleRow
      nopoolhint - drop Pool from For_i hint_engines
    """
    has_ind = "noind" not in variant
    has_cnt = has_ind and "nocnt" not in variant
    has_dil = has_cnt and "nodil" not in variant
    has_mb = has_dil and "nomb" not in variant
    has_sel = has_mb and "nosel" not in variant
    has_conv = "noconv" not in variant
    use_pool = "actrelu" not in variant
    use_dr = "nodr" not in variant
    ind_dt = F8 if use_dr else BF16

    nc = bacc.Bacc("TRN2", target_bir_lowering=False, debug=False,
                   enable_asserts=True, num_devices=NCORES)

    xin = nc.dram_tensor("xin", [NT, 128, G, WP], BF16, kind="ExternalInput")
    pin = nc.dram_tensor("pin", [NT, 128, G, WP], BF16, kind="ExternalInput")
    pout = nc.dram_tensor("pout", [NT, 128, 8 * W], BF16, kind="ExternalInput")
    seldr = nc.dram_tensor("seldr", [128, 5, 2, R], F8, kind="ExternalInput")
    sel10 = nc.dram_tensor("sel10", [128, G * R], BF16, kind="ExternalInput")
    sel2x = nc.dram_tensor("sel2x", [2, 128], BF16, kind="ExternalInput")
    biasv = nc.dram_tensor("biasv", [128, 1], F32, kind="ExternalInput")
    wtbd = nc.dram_tensor("wtbd", [128, 9 * 128], BF16, kind="ExternalInput")
    outd = nc.dram_tensor("out", [NT, 128, 8 * W], BF16, kind="ExternalOutput")

    with tile.TileContext(nc) as tc:
        with tc.tile_pool(name="consts", bufs=1) as cpool, \
             tc.tile_pool(name="io", bufs=3) as iopool, \
             tc.tile_pool(name="mask", bufs=2) as mpool, \
             tc.tile_pool(name="cnt", bufs=2, space="PSUM") as cntpool, \
             tc.tile_pool(name="conv", bufs=4, space="PSUM") as convpool, \
             tc.tile_pool(name="mb", bufs=2, space="PSUM") as mbpool:

            seldrt = cpool.tile([128, 5, 2, R], F8)
            sel10t = cpool.tile([128, G * R], BF16)
            sel2xt = cpool.tile([2, 128], BF16)
            biast = cpool.tile([128, 1], F32)
            wtbdt = cpool.tile([128, 9 * 128], BF16)
            negthr = cpool.tile([128, 1], F32)
            nc.sync.dma_start(out=seldrt[:], in_=seldr[:])
            nc.sync.dma_start(out=sel10t[:], in_=sel10[:])
            nc.sync.dma_start(out=sel2xt[:], in_=sel2x[:])
            nc.sync.dma_start(out=biast[:], in_=biasv[:])
            nc.sync.dma_start(out=wtbdt[:], in_=wtbd[:])
            nc.vector.memset(negthr[:], -THR2)

            def emit_tile(t):
                xt = iopool.tile([128, G, WP], BF16, tag="xt")
                pt = iopool.tile([128, G, WP], BF16, tag="pt")
                pvt = iopool.tile([128, 8 * W], BF16, tag="pvt")
                nc.sync.dma_start(out=xt[:], in_=xin[t])
                nc.sync.dma_start(out=pt[:], in_=pin[t])
                nc.sync.dma_start(out=pvt[:], in_=pout[t])

                dil1 = None
                if has_ind:
                    # --- change indicator: max((x-p)^2 - thr^2, 0) as fp8 ---
                    d = mpool.tile([128, G, WP], BF16, tag="d")
                    nc.vector.tensor_tensor(out=d[:], in0=xt[:], in1=pt[:],
                                            op=mybir.AluOpType.subtract)
                    ind = mpool.tile([128, G, WP], ind_dt, tag="ind")
                    if use_pool:
                        nc.vector.tensor_tensor(out=d[:], in0=d[:], in1=d[:],
                                                op=mybir.AluOpType.mult)
                        nc.gpsimd.tensor_scalar(out=ind[:], in0=d[:],
                                                scalar1=THR2, scalar2=0.0,
                                                op0=mybir.AluOpType.subtract,
                                                op1=mybir.AluOpType.max)
                    elif "dvesq" in variant:
                        nc.vector.tensor_tensor(out=d[:], in0=d[:], in1=d[:],
                                                op=mybir.AluOpType.mult)
                        nc.scalar.activation(ind[:], d[:],
                                             mybir.ActivationFunctionType.Relu,
                                             bias=negthr[:])
                    else:
                        nc.scalar.activation(d[:], d[:],
                                             mybir.ActivationFunctionType.Square)
                        nc.scalar.activation(ind[:], d[:],
                                             mybir.ActivationFunctionType.Relu,
                                             bias=negthr[:])

                if has_cnt:
                    # --- change count + H-dilation via banded matmuls ---
                    # fp8 DoubleRow: each MM contracts 2 row-slots x 128
                    # partitions (rows 8,9 counted twice -- harmless, only
                    # nonzero-ness is used). Band weights are 1/64 so dil
                    # values stay well inside fp8 range.
                    cnt = cntpool.tile([R, W], F32, tag="cnt")
                    if use_dr:
                        for k in range(5):
                            nc.tensor.matmul(
                                cnt[:], seldrt[:, k],
                                ind[:, 2 * k:2 * k + 2, 1:1 + W],
                                start=(k == 0), stop=(k == 4),
                                perf_mode=mybir.MatmulPerfMode.DoubleRow)
                    else:
                        for k in range(G):
                            nc.tensor.matmul(
                                cnt[:], sel10t[:, k * R:(k + 1) * R],
                                ind[:, k, 1:1 + W],
                                start=(k == 0), stop=(k == G - 1))

                if has_dil:
                    # --- W-dilation on [R, W+2] ---
                    hs = mpool.tile([R, WP], F32, tag="hs")
                    nc.vector.memset(hs[:], 0.0)
                    nc.vector.tensor_copy(out=hs[:, 1:W + 1], in_=cnt[:])
                    t1 = mpool.tile([R, W + 1], F32, tag="t1")
                    nc.vector.tensor_tensor(out=t1[:], in0=hs[:, 0:W + 1],
                                            in1=hs[:, 1:WP],
                                            op=mybir.AluOpType.add)
                    dil = mpool.tile([R, W], BF16, tag="dil")
                    nc.vector.tensor_tensor(out=dil[:], in0=t1[:, 0:W],
                                            in1=hs[:, 2:WP],
                                            op=mybir.AluOpType.add)
                    dil1 = mpool.tile([2, 8 * W], BF16, tag="dil1")
                    dma_eng = nc.scalar if "sdma" in variant else nc.vector
                    dma_eng.dma_start(out=dil1[:], in_=dil[:])

                conv_sb = iopool.tile([128, 8 * W], BF16, tag="conv_sb")
                for j in range(8):
                    sl = slice(j * W, (j + 1) * W)
                    if has_conv:
                        # --- conv pair: rows (j, j+8) -> one PSUM bank ---
                        cb = convpool.tile([128, W], F32, tag="cb")
                        # block-diag lhsT [[W,0],[0,W]] computes BOTH halves
                        # of the pair in one 128-contraction MM: partitions
                        # 0:64 = group0 row j+dh -> out 0:64; partitions
                        # 64:128 = group1 row j+dh (= row 8+j+dh) -> 64:128.
                        taps = [(dh, dw) for dh in range(3) for dw in range(3)]
                        for i, (dh, dw) in enumerate(taps):
                            ti = dh * 3 + dw
                            nc.tensor.matmul(
                                cb[:],
                                wtbdt[:, ti * 128:(ti + 1) * 128],
                                xt[:, j + dh, dw:dw + W],
                                start=(i == 0), stop=(i == len(taps) - 1))
                        # --- evacuate conv + bias (fp32 psum -> bf16) ---
                        nc.scalar.activation(
                            conv_sb[:, sl], cb[:],
                            mybir.ActivationFunctionType.Identity,
                            bias=biast[:])

                for j in range(8):
                    sl = slice(j * W, (j + 1) * W)
                    if has_mb:
                        # --- broadcast dilated counts for rows (j, j+8) ---
                        mb = mbpool.tile([128, W], F32, tag="mb")
                        nc.tensor.matmul(mb[:], sel2xt[:],
                                         dil1[:, j * W:(j + 1) * W],
                                         start=True, stop=True)

                    # --- merge conv over prev_output ---
                    if has_sel and has_conv:
                        nc.vector.copy_predicated(
                            pvt[:, sl], mb[:].bitcast(mybir.dt.int32),
                            conv_sb[:, sl])
                    elif has_conv:
                        nc.vector.tensor_copy(out=pvt[:, sl],
                                              in_=conv_sb[:, sl])

                (nc.scalar if "sdma" in variant else nc.vector).dma_start(
                    out=outd[t], in_=pvt[:])

            hints = [mybir.EngineType.PE, mybir.EngineType.DVE,
                     mybir.EngineType.Activation, mybir.EngineType.SP]
            if use_pool and has_ind and "nopoolhint" not in variant:
                hints.insert(3, mybir.EngineType.Pool)
            unroll = 1
            for tok in variant.split(","):
                if tok.startswith("u") and tok[1:].isdigit():
                    unroll = int(tok[1:])
            if loop_iters > 0:
                def body(iv):
                    for t in range(NT):
                        emit_tile(t)
                if unroll > 1:
                    tc.For_i_unrolled(0, loop_iters, 1, body, unroll)
                else:
                    with tc.For_i(0, loop_iters, 1, hint_engines=tuple(hints),
                                  staggered_reset="sr" in variant.split(",")):
                        body(0)
            else:
                for t in range(NT):
                    emit_tile(t)

    nc.compile()
    return nc


def host_prep(inp, prev_input, prev_output, weight, bias):
    """Build per-core in_maps."""
    inp = np.asarray(inp)
    prev_input = np.asarray(prev_input)
    prev_output = np.asarray(prev_output)
    weight = np.asarray(weight)
    bias = np.asarray(bias)

    xpad = np.zeros((C, H + 2, WP), dtype=BF)
    ppad = np.zeros((C, H + 2, WP), dtype=BF)
    xpad[:, 1:H + 1, 1:W + 1] = inp[0].astype(BF)
    ppad[:, 1:H + 1, 1:W + 1] = prev_input[0].astype(BF)

    # sel bands: group0 handles padded rows p=0..9 (slot k=p), group1
    # p=10..17 (slot k=p-8). band(k)[u] = 1/64 if k-2 <= u <= k, 0 <= u < R.
    selA = np.zeros((G, R), dtype=np.float32)
    selB = np.zeros((G, R), dtype=np.float32)
    for p in range(0, 10):
        for u in range(max(0, p - 2), min(R, p + 1)):
            selA[p, u] = 1.0 / 64
    for p in range(10, NPAD):
        for u in range(max(0, p - 2), min(R, p + 1)):
            selB[p - 8, u] = 1.0 / 64
    seldr = np.empty((128, 5, 2, R), dtype=F8NP)
    for k in range(G):
        seldr[:64, k // 2, k % 2] = selA[k].astype(F8NP)
        seldr[64:, k // 2, k % 2] = selB[k].astype(F8NP)
    sel10 = np.empty((128, G * R), dtype=BF)
    sel10[:64] = selA.astype(BF).reshape(1, G * R)
    sel10[64:] = selB.astype(BF).reshape(1, G * R)

    wtbd = np.zeros((128, 9 * 128), dtype=BF)
    for dh in range(3):
        for dw in range(3):
            ti = dh * 3 + dw
            wtap_ = weight[:, :, dh, dw].T.astype(BF)   # [ci, co]
            wtbd[0:64, ti * 128:ti * 128 + 64] = wtap_
            wtbd[64:128, ti * 128 + 64:(ti + 1) * 128] = wtap_

    sel2x = np.zeros((2, 128), dtype=BF)
    sel2x[0, :64] = 1
    sel2x[1, 64:] = 1
    biasv = np.tile(bias.astype(np.float32).reshape(-1, 1), (2, 1))  # [128,1]

    in_maps = []
    for c in range(NCORES):
        r0 = c * RPC

        def slab(pad):
            s = np.empty((NT, 128, G, WP), dtype=BF)
            for t in range(NT):
                rows = pad[:, r0 + 16 * t: r0 + 16 * t + NPAD, :]  # [C,18,WP]
                s[t, :64] = rows[:, 0:10]
                s[t, 64:] = rows[:, 8:18]
            return s

        po = prev_output[0][:, r0:r0 + RPC, :].reshape(C, NT, 2, 8, W)
        po = po.transpose(1, 2, 0, 3, 4).reshape(NT, 128, 8 * W)
        po = np.ascontiguousarray(po).astype(BF)

        in_maps.append({
            "xin": slab(xpad), "pin": slab(ppad), "pout": po,
            "seldr": seldr, "sel10": sel10, "sel2x": sel2x,
            "biasv": biasv, "wtbd": wtbd,
        })
    return in_maps


def host_post(results):
    """Reassemble [NCORES] x [NT, 128, 8*W] bf16 -> [1, C, H, W] fp32."""
    out = np.empty((1, C, H, W), dtype=np.float32)
    for c, res in enumerate(results):
        o = res["out"].astype(np.float32)
        o = o.reshape(NT, 2, C, 8, W).transpose(2, 0, 1, 3, 4)
        out[0, :, c * RPC:(c + 1) * RPC, :] = o.reshape(C, RPC, W)
    return out


def kernel(inp, prev_input, prev_output, weight, bias):
    if "nc" not in _cached:
        _cached["nc"] = build_nc(0)
    nc = _cached["nc"]
    in_maps = host_prep(inp, prev_input, prev_output, weight, bias)
    res = run_bass_kernel_spmd(nc, in_maps, core_ids=list(range(NCORES)))
    return host_post(res.results)


if __name__ == "__main__":
    rng = np.random.default_rng(0)
    inp = rng.standard_normal((1, C, H, W), dtype=np.float32)
    prev_input = inp + 0.05 * rng.standard_normal((1, C, H, W), dtype=np.float32)
    prev_output = rng.standard_normal((1, C, H, W), dtype=np.float32)
    weight = (0.05 * rng.standard_normal((C, C, 3, 3))).astype(np.float32)
    bias = rng.standard_normal(C).astype(np.float32)
    out = kernel(inp=inp, prev_input=prev_input, prev_output=prev_output,
                 weight=weight, bias=bias)
    print("out", out.shape, out.dtype, float(np.abs(out).mean()))
